# revision 14
# baseline (speedup 1.0000x reference)
"""Causal self-attention (B=4, T=2048, C=1024, H=16) on 8 TRN2 NeuronCores.

Sharding: tensor-parallel over heads - 2 heads per core. Each core:
  - computes Q^T,K^T (head-dim on partitions) and V (token-dim on partitions)
    for its 2 heads from the full input x,
  - runs causal attention head-sequentially in transposed-score layout
    S^T[k, q] with 1024-wide q chunks: each S tile is a [128, 1024] f32
    2-PSUM-bank tile so softmax exp is ONE wide activation per k-tile,
  - the softmax denominator comes from a ones-column appended to V; per-head
    column placement (h0: hd at 0:64 + ones at 64; h1: ones at 63 + hd at
    64:128) lands each head's output on its own partitions so normalized
    y is written in place,
  - 1/l is computed as Exp(-Ln(l)) on the scalar engine (both functions in
    the natural_log_exp table set, so no table switching) and broadcast to
    64 partitions via the idle GPSIMD engine,
  - computes a partial output  y_local @ w_proj[:, c_slice]^T  over its 128
    channels.
Host sums the 8 partials (the all-reduce of the row-sharded projection).

Matmuls run in bf16 (fp32 PSUM accumulation); softmax exp runs in fp32 on
the scalar engine without max-subtraction (scores are O(4) after the 1/8
scale, far below fp32 overflow).
"""

import numpy as np
import ml_dtypes

B, T, C, H = 4, 2048, 1024, 16
HD = C // H            # 64 head dim
NCORES = 8
HPC = H // NCORES      # 2 heads per core
RPC = HPC * HD         # 128 rows (channels) per core for each of q/k/v
BT = B * T             # 8192
CT = C // 128          # 8 contraction tiles
QCH = 1024             # q-chunk width (2 psum banks of f32)
NCH = T // QCH         # 2 chunks per (b, h)
NTT = T // 128         # 16 token tiles per batch

_prog_cache = {}


def build_program():
    """Build the (SPMD-identical) Bass program. Inputs differ per core."""
    from contextlib import ExitStack
    import concourse.bass as bass
    import concourse.mybir as mybir
    import concourse.tile as tile
    from concourse import bacc

    f32 = mybir.dt.float32
    bf16 = mybir.dt.bfloat16
    f16 = mybir.dt.float16
    EXP = mybir.ActivationFunctionType.Exp
    LN = mybir.ActivationFunctionType.Ln

    nc = bacc.Bacc("TRN2", target_bir_lowering=False, debug=False)

    xt = nc.dram_tensor("xt", [CT, B, 128, T], bf16, kind="ExternalInput").ap()
    wqkv = nc.dram_tensor("wqkv", [CT, 128, 3 * RPC], bf16, kind="ExternalInput").ap()
    wproj = nc.dram_tensor("wproj", [128, C], bf16, kind="ExternalInput").ap()
    outp = nc.dram_tensor("outp", [BT, C], bf16, kind="ExternalOutput").ap()

    with tile.TileContext(nc) as tc, ExitStack() as ctx:
        const = ctx.enter_context(tc.tile_pool(name="const", bufs=1))
        qk_pool = ctx.enter_context(tc.tile_pool(name="qkp", bufs=2))
        v_pool = ctx.enter_context(tc.tile_pool(name="vp", bufs=2))
        pt_pool = ctx.enter_context(tc.tile_pool(name="ptp", bufs=3))
        nrm_pool = ctx.enter_context(tc.tile_pool(name="nrm", bufs=1))
        rb_pool = ctx.enter_context(tc.tile_pool(name="rbp", bufs=2))
        yu_pool = ctx.enter_context(tc.tile_pool(name="yup", bufs=2))
        st_pool = ctx.enter_context(tc.tile_pool(name="stp", bufs=2))
        # PSUM: S tiles (2 banks x 2 bufs) | Y accumulator (2 banks) | qkv/
        # vtr/proj rotating pool (2 banks)
        ps_s = ctx.enter_context(tc.tile_pool(name="pss", bufs=2, space="PSUM"))
        ps_y = ctx.enter_context(tc.tile_pool(name="psy", bufs=1, space="PSUM"))
        ps_q = ctx.enter_context(tc.tile_pool(name="psq", bufs=2, space="PSUM"))

        # ---- constants ----
        wqkv_sb = const.tile([128, CT, 3 * RPC], bf16, tag="wqkv")
        nc.sync.dma_start(out=wqkv_sb, in_=wqkv.rearrange("ct p r -> p ct r"))
        wproj_sb = const.tile([128, C], bf16, tag="wproj")
        nc.sync.dma_start(out=wproj_sb, in_=wproj)

        ident = const.tile([128, 128], bf16, tag="ident")
        from concourse.masks import make_identity
        make_identity(nc, ident)

        # stage all of x^T in SBUF once (128KB/partition) - each region is
        # written exactly once so no DMA ever carries a WAR/WAW wait.
        xt_sb = const.tile([128, B, CT, T], bf16, tag="xts")
        for b in range(B):
            for c in range(CT):
                nc.sync.dma_start(out=xt_sb[:, b, c, :], in_=xt[c, b])

        # tri[p, f] = 1.0 where p <= f else 0 (keep k <= q in transposed scores)
        tri = const.tile([128, 128], bf16, tag="tri")
        nc.gpsimd.memset(tri, 1.0)
        nc.gpsimd.affine_select(
            out=tri, in_=tri,
            compare_op=mybir.AluOpType.is_ge,
            fill=0.0, base=0,
            channel_multiplier=-1,       # expr = -p + f >= 0  -> keep
            pattern=[[1, 128]],
        )

        st = {}   # per-b tiles

        def emit_qkv_unit(b, q5):
            """QKV projection for one 512-col t-chunk + V transposes for it."""
            if q5 == 0:
                qt_b = qk_pool.tile([128, T], bf16, tag="qt", name=f"qt_{b}")
                kt_b = qk_pool.tile([128, T], bf16, tag="kt", name=f"kt_{b}")
                vt_b = qk_pool.tile([128, T], bf16, tag="vt", name=f"vt_{b}")
                yl_b = qk_pool.tile([128, T], bf16, tag="yl", name=f"yl_{b}")
                v_b = v_pool.tile([128, NTT, HPC, 128], bf16, tag="v",
                                  name=f"v_{b}")
                # h0: hd at cols 0:64, ones at col 64, zeros above.
                # h1: ones at col 0 (partition-aligned l row), hd at 64:128.
                nc.vector.memset(v_b[:, :, 0, HD:], 0.0)
                nc.vector.memset(v_b[:, :, 0, HD:HD + 1], 1.0)
                nc.vector.memset(v_b[:, :, 1, :HD], 0.0)
                nc.vector.memset(v_b[:, :, 1, 0:1], 1.0)
                st[b] = (qt_b, kt_b, vt_b, yl_b, v_b)
            qt_b, kt_b, vt_b, yl_b, v_b = st[b]
            for rg, dest in ((0, qt_b), (1, kt_b), (2, vt_b)):
                acc = ps_q.tile([128, 512], f32, tag="q",
                                name=f"qkv_{b}_{rg}_{q5}")
                for c in range(CT):
                    nc.tensor.matmul(
                        acc,
                        lhsT=wqkv_sb[:, c, rg * 128:(rg + 1) * 128],
                        rhs=xt_sb[:, b, c, q5 * 512:(q5 + 1) * 512],
                        start=(c == 0), stop=(c == CT - 1),
                    )
                nc.vector.tensor_copy(dest[:, q5 * 512:(q5 + 1) * 512], acc)
            for tt in range(4 * q5, 4 * q5 + 4):
                vtr = ps_q.tile([128, 128], bf16, tag="q", name=f"vtr_{b}_{tt}")
                nc.tensor.transpose(vtr, vt_b[:, tt * 128:(tt + 1) * 128], ident)
                nc.vector.tensor_copy(v_b[:, tt, 0, 0:HD], vtr[:, 0:HD])
                nc.vector.tensor_copy(v_b[:, tt, 1, HD:128], vtr[:, HD:128])

        def emit_attn_chunk(b, h, ch):
            """Causal attention for head h, q-chunk ch (1024 wide)."""
            qt_b, kt_b, vt_b, yl_b, v_b = st[b]
            hp = h * HD
            q0 = ch * QCH
            nkt = 8 * (ch + 1)
            alast = 3 + 8 * ch        # last j contributing to cols [0:512)
            yaug = ps_y.tile([128, QCH], f32, tag="y", name=f"yaug_{b}_{h}_{ch}")

            def consume(j, s, lo):
                p = pt_pool.tile([128, QCH], bf16, tag="p",
                                 name=f"p_{b}_{h}_{ch}_{j}")
                nc.scalar.activation(p[:, lo:QCH], s[:, lo:QCH], EXP,
                                     scale=1.0 / 8.0)
                dl = 128 * j - QCH * ch
                if dl >= 0:   # diagonal block [dl, dl+128) needs causal mask
                    # GPSIMD: idle engine, so the mask never queues behind
                    # the DVE's evacuation work in the S->exp->PV chain.
                    nc.gpsimd.tensor_mul(p[:, dl:dl + 128], p[:, dl:dl + 128],
                                         tri)
                if lo < 512:
                    nc.tensor.matmul(
                        yaug[:, lo:512],
                        lhsT=v_b[:, j, h, :],
                        rhs=p[:, lo:512],
                        start=(j == 0), stop=(j == alast),
                    )
                b0 = max(lo, 512)
                nc.tensor.matmul(
                    yaug[:, b0:QCH],
                    lhsT=v_b[:, j, h, :],
                    rhs=p[:, b0:QCH],
                    start=(j == 0), stop=(j == nkt - 1),
                )

            prev = None
            for j in range(nkt):
                lo = max(0, 128 * j - QCH * ch)
                s = ps_s.tile([128, QCH], f32, tag="s",
                              name=f"s_{b}_{h}_{ch}_{j}")
                if lo < 512:
                    nc.tensor.matmul(
                        s[:, lo:512],
                        lhsT=kt_b[hp:hp + HD, j * 128:(j + 1) * 128],
                        rhs=qt_b[hp:hp + HD, q0 + lo:q0 + 512],
                        start=True, stop=True,
                    )
                b0 = max(lo, 512)
                nc.tensor.matmul(
                    s[:, b0:QCH],
                    lhsT=kt_b[hp:hp + HD, j * 128:(j + 1) * 128],
                    rhs=qt_b[hp:hp + HD, q0 + b0:q0 + QCH],
                    start=True, stop=True,
                )
                if prev is not None:
                    consume(*prev)
                prev = (j, s, lo)
            consume(*prev)

            # Evacuate yaug early (one full-height CAST frees both PSUM banks
            # in ~1.2us), then normalize entirely off the PE/ACT engines:
            # broadcast raw l with GPSIMD, 1/l via int bit-trick + one
            # Newton step on DVE, multi-lane.
            yu = yu_pool.tile([128, QCH], bf16, tag="yu",
                              name=f"yu_{b}_{h}_{ch}")
            nc.scalar.copy(yu, yaug)
            rb_sb = rb_pool.tile([128, QCH], bf16, tag="rb",
                                 name=f"rb_{b}_{h}_{ch}")
            if h == 0:
                # l sits at partition 64; partition_broadcast needs src at
                # partition 0 -> tiny SBUF->SBUF DMA row shift first.
                nc.sync.dma_start(out=rb_sb[0:1, :], in_=yu[HD:HD + 1, :])
                nc.gpsimd.partition_broadcast(rb_sb[0:HD, :], rb_sb[0:1, :])
                rows = slice(0, HD)
            else:
                nc.gpsimd.partition_broadcast(rb_sb, yu[0:1, :])
                rows = slice(HD, 128)
            i16 = mybir.dt.int16
            y0 = nrm_pool.tile([128, QCH], bf16, tag="y0",
                               name=f"y0_{b}_{h}_{ch}")
            nc.vector.tensor_scalar(
                y0[rows, :].bitcast(i16), rb_sb[rows, :].bitcast(i16),
                -1.0, float(0x7EF7), mybir.AluOpType.mult, mybir.AluOpType.add)
            tn = nrm_pool.tile([128, QCH], f16, tag="tn",
                               name=f"tn_{b}_{h}_{ch}")
            nc.vector.tensor_mul(tn[rows, :], y0[rows, :], rb_sb[rows, :])
            nc.vector.tensor_scalar(
                tn[rows, :], tn[rows, :], -1.0, 2.0,
                mybir.AluOpType.mult, mybir.AluOpType.add)
            nc.vector.tensor_mul(rb_sb[rows, :], y0[rows, :], tn[rows, :])
            nc.vector.tensor_mul(yl_b[hp:hp + HD, q0:q0 + QCH],
                                 yu[rows, :], rb_sb[rows, :])

        def emit_proj(b, ch):
            """Output projection for the 8 token-tiles of q-chunk ch."""
            yl_b = st[b][3]
            for tt in range(8 * ch, 8 * ch + 8):
                for n5 in range(C // 512):
                    op = ps_q.tile([128, 512], f32, tag="q",
                                   name=f"op_{b}_{tt}_{n5}")
                    nc.tensor.matmul(
                        op,
                        lhsT=yl_b[:, tt * 128:(tt + 1) * 128],
                        rhs=wproj_sb[:, n5 * 512:(n5 + 1) * 512],
                        start=True, stop=True,
                    )
                    o_sb = st_pool.tile([128, 512], bf16, tag="o",
                                        name=f"o_{b}_{tt}_{n5}")
                    if tt % 4 == 3:
                        nc.scalar.copy(o_sb, op)
                    else:
                        nc.vector.tensor_copy(o_sb, op)
                    nc.sync.dma_start(
                        out=outp[b * T + tt * 128: b * T + (tt + 1) * 128,
                                 n5 * 512:(n5 + 1) * 512],
                        in_=o_sb)

        # software pipeline: QKV of b+1 and proj of b interleave with attn of b
        for q5 in range(4):
            emit_qkv_unit(0, q5)
        for b in range(B):
            fill = iter(range(4))
            for h in range(HPC):
                for ch in range(NCH):
                    emit_attn_chunk(b, h, ch)
                    if b + 1 < B:
                        emit_qkv_unit(b + 1, next(fill))
                    if h == 1:
                        emit_proj(b, ch)
            del st[b]

    nc.compile()
    return nc


def _prep_inputs(x, w_attn, w_proj):
    """Host-side sharding: build per-core input maps."""
    bf16 = ml_dtypes.bfloat16
    x = np.asarray(x, dtype=np.float32)
    w_attn = np.asarray(w_attn, dtype=np.float32)
    w_proj = np.asarray(w_proj, dtype=np.float32)

    # x^T tiles: [CT, B, 128, T]
    xt = np.ascontiguousarray(
        x.reshape(BT, C).T.reshape(CT, 128, B, T).transpose(0, 2, 1, 3)
    ).astype(bf16)

    in_maps = []
    for g in range(NCORES):
        r0 = g * RPC
        w_local = np.concatenate([
            w_attn[r0:r0 + RPC],              # q rows of heads 2g, 2g+1
            w_attn[C + r0:C + r0 + RPC],      # k rows
            w_attn[2 * C + r0:2 * C + r0 + RPC],  # v rows
        ], axis=0)                            # [384, C]
        wqkv = np.ascontiguousarray(
            w_local.T.reshape(CT, 128, 3 * RPC)).astype(bf16)
        wprojT = np.ascontiguousarray(w_proj[:, r0:r0 + RPC].T).astype(bf16)
        in_maps.append({"xt": xt, "wqkv": wqkv, "wproj": wprojT})
    return in_maps


def kernel(x, w_attn, w_proj):
    from concourse import bass_utils

    if "nc" not in _prog_cache:
        _prog_cache["nc"] = build_program()
    nc = _prog_cache["nc"]

    in_maps = _prep_inputs(x, w_attn, w_proj)
    res = bass_utils.run_bass_kernel_spmd(
        nc, in_maps, core_ids=list(range(NCORES)))

    acc = np.zeros((BT, C), dtype=np.float32)
    for g in range(NCORES):
        part = np.asarray(res.results[g]["outp"])
        if part.dtype != np.float32:
            # bf16 -> f32 exact upcast via bit manipulation (fast on host)
            part = (part.view(np.uint16).astype(np.uint32) << 16).view(np.float32)
        acc += part
    return acc.reshape(B, T, C)


# revision 21
# speedup vs baseline: 1.3339x; 1.3339x over previous
"""Causal self-attention (B=4, T=2048, C=1024, H=16) on 8 TRN2 NeuronCores.

Sharding: tensor-parallel over heads - 2 heads per core. Each core:
  - computes Q^T,K^T (head-dim on partitions) and V (token-dim on partitions)
    for its 2 heads from the full input x,
  - runs causal attention head-sequentially in transposed-score layout
    S^T[k, q] with 1024-wide q chunks: each S tile is a [128, 1024] f32
    2-PSUM-bank tile so softmax exp is ONE wide activation per k-tile,
  - the softmax denominator comes from a ones-column appended to V; per-head
    column placement (h0: hd at 0:64 + ones at 64; h1: ones at 63 + hd at
    64:128) lands each head's output on its own partitions so normalized
    y is written in place,
  - 1/l is computed as Exp(-Ln(l)) on the scalar engine (both functions in
    the natural_log_exp table set, so no table switching) and broadcast to
    64 partitions via the idle GPSIMD engine,
  - computes a partial output  y_local @ w_proj[:, c_slice]^T  over its 128
    channels.
Host sums the 8 partials (the all-reduce of the row-sharded projection).

Matmuls run in bf16 (fp32 PSUM accumulation); softmax exp runs in fp32 on
the scalar engine without max-subtraction (scores are O(4) after the 1/8
scale, far below fp32 overflow).
"""

import numpy as np
import ml_dtypes

B, T, C, H = 4, 2048, 1024, 16
HD = C // H            # 64 head dim
NCORES = 8
HPC = H // NCORES      # 2 heads per core
RPC = HPC * HD         # 128 rows (channels) per core for each of q/k/v
BT = B * T             # 8192
CT = C // 128          # 8 contraction tiles
QCH = 1024             # q-chunk width (2 psum banks of f32)
NCH = T // QCH         # 2 chunks per (b, h)
NTT = T // 128         # 16 token tiles per batch

_prog_cache = {}


def build_program():
    """Build the (SPMD-identical) Bass program. Inputs differ per core."""
    from contextlib import ExitStack
    import concourse.bass as bass
    import concourse.mybir as mybir
    import concourse.tile as tile
    from concourse import bacc

    f32 = mybir.dt.float32
    bf16 = mybir.dt.bfloat16
    f16 = mybir.dt.float16
    EXP = mybir.ActivationFunctionType.Exp
    LN = mybir.ActivationFunctionType.Ln

    nc = bacc.Bacc("TRN2", target_bir_lowering=False, debug=False)

    xt = nc.dram_tensor("xt", [CT, B, 128, T], bf16, kind="ExternalInput").ap()
    wqkv = nc.dram_tensor("wqkv", [CT, 128, 3 * RPC], bf16, kind="ExternalInput").ap()
    wproj = nc.dram_tensor("wproj", [128, C], bf16, kind="ExternalInput").ap()
    outp = nc.dram_tensor("outp", [BT, C], bf16, kind="ExternalOutput").ap()

    with tile.TileContext(nc) as tc, ExitStack() as ctx:
        const = ctx.enter_context(tc.tile_pool(name="const", bufs=1))
        qk_pool = ctx.enter_context(tc.tile_pool(name="qkp", bufs=2))
        v_pool = ctx.enter_context(tc.tile_pool(name="vp", bufs=2))
        pt_pool = ctx.enter_context(tc.tile_pool(name="ptp", bufs=3))
        nrm_pool = ctx.enter_context(tc.tile_pool(name="nrm", bufs=1))
        rb_pool = ctx.enter_context(tc.tile_pool(name="rbp", bufs=2))
        yu_pool = ctx.enter_context(tc.tile_pool(name="yup", bufs=2))
        st_pool = ctx.enter_context(tc.tile_pool(name="stp", bufs=2))
        # PSUM: S tiles (2 banks x 2 bufs) | Y accumulator (2 banks) | qkv/
        # vtr/proj rotating pool (2 banks)
        ps_s = ctx.enter_context(tc.tile_pool(name="pss", bufs=2, space="PSUM"))
        ps_y = ctx.enter_context(tc.tile_pool(name="psy", bufs=1, space="PSUM"))
        ps_q = ctx.enter_context(tc.tile_pool(name="psq", bufs=2, space="PSUM"))

        # ---- constants ----
        wqkv_sb = const.tile([128, CT, 3 * RPC], bf16, tag="wqkv")
        nc.sync.dma_start(out=wqkv_sb, in_=wqkv.rearrange("ct p r -> p ct r"))
        wproj_sb = const.tile([128, C], bf16, tag="wproj")
        nc.sync.dma_start(out=wproj_sb, in_=wproj)

        ident = const.tile([128, 128], bf16, tag="ident")
        from concourse.masks import make_identity
        make_identity(nc, ident)

        # stage all of x^T in SBUF once (128KB/partition) - each region is
        # written exactly once so no DMA ever carries a WAR/WAW wait.
        xt_sb = const.tile([128, B, CT, T], bf16, tag="xts")
        for b in range(B):
            for c in range(CT):
                nc.sync.dma_start(out=xt_sb[:, b, c, :], in_=xt[c, b])

        # tri[p, f] = 1.0 where p <= f else 0 (keep k <= q in transposed scores)
        tri = const.tile([128, 128], bf16, tag="tri")
        nc.gpsimd.memset(tri, 1.0)
        nc.gpsimd.affine_select(
            out=tri, in_=tri,
            compare_op=mybir.AluOpType.is_ge,
            fill=0.0, base=0,
            channel_multiplier=-1,       # expr = -p + f >= 0  -> keep
            pattern=[[1, 128]],
        )

        st = {}   # per-b tiles

        def _make_tiles(b):
            qt_b = qk_pool.tile([128, T], bf16, tag="qt", name=f"qt_{b}")
            kt_b = qk_pool.tile([128, T], bf16, tag="kt", name=f"kt_{b}")
            vt_b = qk_pool.tile([128, T], bf16, tag="vt", name=f"vt_{b}")
            yl_b = qk_pool.tile([128, T], bf16, tag="yl", name=f"yl_{b}")
            v_b = v_pool.tile([128, NTT, HPC, 128], bf16, tag="v",
                              name=f"v_{b}")
            # h0: hd at cols 0:64, ones at col 64, zeros above.
            # h1: ones at col 0 (partition-aligned l row), hd at 64:128.
            nc.vector.memset(v_b[:, :, 0, HD:], 0.0)
            nc.vector.memset(v_b[:, :, 0, HD:HD + 1], 1.0)
            nc.vector.memset(v_b[:, :, 1, :HD], 0.0)
            nc.vector.memset(v_b[:, :, 1, 0:1], 1.0)
            st[b] = (qt_b, kt_b, vt_b, yl_b, v_b)

        def qkv_thunks(b):
            """One thunk per matmul of b's QKV projection (+V transposes),
            pulled into the attention j-loop as deterministic PE filler."""
            thunks = []

            def ensure():
                if b not in st:
                    _make_tiles(b)

            accs = {}

            def mm(rg, q5, c):
                ensure()
                qt_b, kt_b, vt_b, yl_b, v_b = st[b]
                if c == 0:
                    accs[(rg, q5)] = ps_q.tile(
                        [128, 512], f32, tag="q", name=f"qkv_{b}_{rg}_{q5}")
                acc = accs[(rg, q5)]
                nc.tensor.matmul(
                    acc,
                    lhsT=wqkv_sb[:, c, rg * 128:(rg + 1) * 128],
                    rhs=xt_sb[:, b, c, q5 * 512:(q5 + 1) * 512],
                    start=(c == 0), stop=(c == CT - 1),
                )
                if c == CT - 1:
                    dest = (qt_b, kt_b, vt_b)[rg]
                    nc.vector.tensor_copy(
                        dest[:, q5 * 512:(q5 + 1) * 512], acc)
                    del accs[(rg, q5)]

            def tr(tt):
                qt_b, kt_b, vt_b, yl_b, v_b = st[b]
                vtr = ps_q.tile([128, 128], bf16, tag="q", name=f"vtr_{b}_{tt}")
                nc.tensor.transpose(vtr, vt_b[:, tt * 128:(tt + 1) * 128],
                                    ident)
                nc.vector.tensor_copy(v_b[:, tt, 0, 0:HD], vtr[:, 0:HD])
                nc.vector.tensor_copy(v_b[:, tt, 1, HD:128], vtr[:, HD:128])

            from functools import partial
            for rg in range(3):
                for q5 in range(4):
                    for c in range(CT):
                        thunks.append(partial(mm, rg, q5, c))
            for tt in range(NTT):
                thunks.append(partial(tr, tt))
            return thunks

        def emit_attn_chunk(b, h, ch, pull):
            """Causal attention for head h, q-chunk ch (1024 wide)."""
            qt_b, kt_b, vt_b, yl_b, v_b = st[b]
            hp = h * HD
            q0 = ch * QCH
            nkt = 8 * (ch + 1)
            alast = 3 + 8 * ch        # last j contributing to cols [0:512)
            yaug = ps_y.tile([128, QCH], f32, tag="y", name=f"yaug_{b}_{h}_{ch}")

            def consume(j, s, lo):
                p = pt_pool.tile([128, QCH], bf16, tag="p",
                                 name=f"p_{b}_{h}_{ch}_{j}")
                nc.scalar.activation(p[:, lo:QCH], s[:, lo:QCH], EXP,
                                     scale=1.0 / 8.0)
                dl = 128 * j - QCH * ch
                if dl >= 0:   # diagonal block [dl, dl+128) needs causal mask
                    nc.vector.tensor_mul(p[:, dl:dl + 128], p[:, dl:dl + 128],
                                         tri)
                if lo < 512:
                    nc.tensor.matmul(
                        yaug[:, lo:512],
                        lhsT=v_b[:, j, h, :],
                        rhs=p[:, lo:512],
                        start=(j == 0), stop=(j == alast),
                    )
                b0 = max(lo, 512)
                nc.tensor.matmul(
                    yaug[:, b0:QCH],
                    lhsT=v_b[:, j, h, :],
                    rhs=p[:, b0:QCH],
                    start=(j == 0), stop=(j == nkt - 1),
                )

            prev = None
            for j in range(nkt):
                lo = max(0, 128 * j - QCH * ch)
                s = ps_s.tile([128, QCH], f32, tag="s",
                              name=f"s_{b}_{h}_{ch}_{j}")
                if lo < 512:
                    nc.tensor.matmul(
                        s[:, lo:512],
                        lhsT=kt_b[hp:hp + HD, j * 128:(j + 1) * 128],
                        rhs=qt_b[hp:hp + HD, q0 + lo:q0 + 512],
                        start=True, stop=True,
                    )
                b0 = max(lo, 512)
                nc.tensor.matmul(
                    s[:, b0:QCH],
                    lhsT=kt_b[hp:hp + HD, j * 128:(j + 1) * 128],
                    rhs=qt_b[hp:hp + HD, q0 + b0:q0 + QCH],
                    start=True, stop=True,
                )
                pull(2)   # deterministic PE filler between S(j) and PV(j-1)
                if prev is not None:
                    consume(*prev)
                prev = (j, s, lo)
            consume(*prev)

            # Evacuate yaug early (one full-height CAST frees both PSUM banks
            # in ~1.2us), then normalize entirely off the PE/ACT engines:
            # broadcast raw l with GPSIMD, 1/l via int bit-trick + one
            # Newton step on DVE, multi-lane.
            yu = yu_pool.tile([128, QCH], bf16, tag="yu",
                              name=f"yu_{b}_{h}_{ch}")
            nc.scalar.copy(yu, yaug)
            rb_sb = rb_pool.tile([128, QCH], bf16, tag="rb",
                                 name=f"rb_{b}_{h}_{ch}")
            if h == 0:
                # l sits at partition 64; partition_broadcast needs src at
                # partition 0 -> tiny SBUF->SBUF DMA row shift first.
                nc.sync.dma_start(out=rb_sb[0:1, :], in_=yu[HD:HD + 1, :])
                nc.gpsimd.partition_broadcast(rb_sb[0:HD, :], rb_sb[0:1, :])
                rows = slice(0, HD)
            else:
                nc.gpsimd.partition_broadcast(rb_sb, yu[0:1, :])
                rows = slice(HD, 128)
            i16 = mybir.dt.int16
            y0 = nrm_pool.tile([128, QCH], bf16, tag="y0",
                               name=f"y0_{b}_{h}_{ch}")
            nc.vector.tensor_scalar(
                y0[rows, :].bitcast(i16), rb_sb[rows, :].bitcast(i16),
                -1.0, float(0x7EF7), mybir.AluOpType.mult, mybir.AluOpType.add)
            tn = nrm_pool.tile([128, QCH], f16, tag="tn",
                               name=f"tn_{b}_{h}_{ch}")
            nc.vector.tensor_mul(tn[rows, :], y0[rows, :], rb_sb[rows, :])
            # rb = (t - 2)*y0 = -(1/l): the sign is folded out in the host
            # gather (acc -= part), saving one DVE op per chunk.
            nc.vector.scalar_tensor_tensor(
                rb_sb[rows, :], tn[rows, :], 2.0, y0[rows, :],
                mybir.AluOpType.subtract, mybir.AluOpType.mult)
            nc.vector.tensor_mul(yl_b[hp:hp + HD, q0:q0 + QCH],
                                 yu[rows, :], rb_sb[rows, :])

        def emit_proj(b, ch):
            """Output projection for the 8 token-tiles of q-chunk ch."""
            yl_b = st[b][3]
            for tt in range(8 * ch, 8 * ch + 8):
                for n5 in range(C // 512):
                    op = ps_q.tile([128, 512], f32, tag="q",
                                   name=f"op_{b}_{tt}_{n5}")
                    nc.tensor.matmul(
                        op,
                        lhsT=yl_b[:, tt * 128:(tt + 1) * 128],
                        rhs=wproj_sb[:, n5 * 512:(n5 + 1) * 512],
                        start=True, stop=True,
                    )
                    o_sb = st_pool.tile([128, 512], bf16, tag="o",
                                        name=f"o_{b}_{tt}_{n5}")
                    if tt % 4 == 3:
                        nc.scalar.copy(o_sb, op)
                    else:
                        nc.vector.tensor_copy(o_sb, op)
                    nc.sync.dma_start(
                        out=outp[b * T + tt * 128: b * T + (tt + 1) * 128,
                                 n5 * 512:(n5 + 1) * 512],
                        in_=o_sb)

        # software pipeline: QKV of b+1 is pulled as per-matmul filler inside
        # the attention j-loops of b; proj of b interleaves after each chunk.
        for t in qkv_thunks(0):
            t()
        for b in range(B):
            fill = qkv_thunks(b + 1) if b + 1 < B else []
            fi = iter(fill)

            def pull(n):
                for _ in range(n):
                    t = next(fi, None)
                    if t is not None:
                        t()

            for h in range(HPC):
                for ch in range(NCH):
                    emit_attn_chunk(b, h, ch, pull)
                    if h == 1:
                        emit_proj(b, ch)
            pull(len(fill))   # drain leftovers (V transposes)
            del st[b]

    nc.compile()
    return nc


def _prep_inputs(x, w_attn, w_proj):
    """Host-side sharding: build per-core input maps."""
    bf16 = ml_dtypes.bfloat16
    x = np.asarray(x, dtype=np.float32)
    w_attn = np.asarray(w_attn, dtype=np.float32)
    w_proj = np.asarray(w_proj, dtype=np.float32)

    # x^T tiles: [CT, B, 128, T]
    xt = np.ascontiguousarray(
        x.reshape(BT, C).T.reshape(CT, 128, B, T).transpose(0, 2, 1, 3)
    ).astype(bf16)

    in_maps = []
    for g in range(NCORES):
        r0 = g * RPC
        w_local = np.concatenate([
            w_attn[r0:r0 + RPC],              # q rows of heads 2g, 2g+1
            w_attn[C + r0:C + r0 + RPC],      # k rows
            w_attn[2 * C + r0:2 * C + r0 + RPC],  # v rows
        ], axis=0)                            # [384, C]
        wqkv = np.ascontiguousarray(
            w_local.T.reshape(CT, 128, 3 * RPC)).astype(bf16)
        wprojT = np.ascontiguousarray(w_proj[:, r0:r0 + RPC].T).astype(bf16)
        in_maps.append({"xt": xt, "wqkv": wqkv, "wproj": wprojT})
    return in_maps


def kernel(x, w_attn, w_proj):
    from concourse import bass_utils

    if "nc" not in _prog_cache:
        _prog_cache["nc"] = build_program()
    nc = _prog_cache["nc"]

    in_maps = _prep_inputs(x, w_attn, w_proj)
    res = bass_utils.run_bass_kernel_spmd(
        nc, in_maps, core_ids=list(range(NCORES)))

    acc = np.zeros((BT, C), dtype=np.float32)
    for g in range(NCORES):
        part = np.asarray(res.results[g]["outp"])
        if part.dtype != np.float32:
            # bf16 -> f32 exact upcast via bit manipulation (fast on host)
            part = (part.view(np.uint16).astype(np.uint32) << 16).view(np.float32)
        # the kernel folds softmax-denominator division in as -(1/l), so
        # each partial comes out negated
        acc -= part
    return acc.reshape(B, T, C)


# revision 27
# speedup vs baseline: 1.5719x; 1.1785x over previous
"""Causal self-attention (B=4, T=2048, C=1024, H=16) on 8 TRN2 NeuronCores.

Sharding: tensor-parallel over heads - 2 heads per core. Each core:
  - computes Q^T,K^T (head-dim on partitions) and V (token-dim on partitions)
    for its 2 heads from the full input x,
  - runs causal attention head-sequentially in transposed-score layout
    S^T[k, q] with 1024-wide q chunks: each S tile is a [128, 1024] f32
    2-PSUM-bank tile so softmax exp is ONE wide activation per k-tile,
  - the softmax denominator comes from a ones-column appended to V; per-head
    column placement (h0: hd at 0:64 + ones at 64; h1: ones at 63 + hd at
    64:128) lands each head's output on its own partitions so normalized
    y is written in place,
  - 1/l is computed as Exp(-Ln(l)) on the scalar engine (both functions in
    the natural_log_exp table set, so no table switching) and broadcast to
    64 partitions via the idle GPSIMD engine,
  - computes a partial output  y_local @ w_proj[:, c_slice]^T  over its 128
    channels.
Host sums the 8 partials (the all-reduce of the row-sharded projection).

Matmuls run in bf16 (fp32 PSUM accumulation); softmax exp runs in fp32 on
the scalar engine without max-subtraction (scores are O(4) after the 1/8
scale, far below fp32 overflow).
"""

import numpy as np
import ml_dtypes

B, T, C, H = 4, 2048, 1024, 16
HD = C // H            # 64 head dim
NCORES = 8
HPC = H // NCORES      # 2 heads per core
RPC = HPC * HD         # 128 rows (channels) per core for each of q/k/v
BT = B * T             # 8192
CT = C // 128          # 8 contraction tiles
QCH = 1024             # q-chunk width (2 psum banks of f32)
NCH = T // QCH         # 2 chunks per (b, h)
NTT = T // 128         # 16 token tiles per batch

_prog_cache = {}


def build_program():
    """Build the (SPMD-identical) Bass program. Inputs differ per core."""
    from contextlib import ExitStack
    import concourse.bass as bass
    import concourse.mybir as mybir
    import concourse.tile as tile
    from concourse import bacc

    f32 = mybir.dt.float32
    bf16 = mybir.dt.bfloat16
    f16 = mybir.dt.float16
    EXP = mybir.ActivationFunctionType.Exp
    LN = mybir.ActivationFunctionType.Ln

    nc = bacc.Bacc("TRN2", target_bir_lowering=False, debug=False)

    xt = nc.dram_tensor("xt", [CT, B, 128, T], bf16, kind="ExternalInput").ap()
    wqkv = nc.dram_tensor("wqkv", [CT, 128, 3 * RPC], bf16, kind="ExternalInput").ap()
    wproj = nc.dram_tensor("wproj", [128, C], bf16, kind="ExternalInput").ap()
    outp = nc.dram_tensor("outp", [BT, C], bf16, kind="ExternalOutput").ap()

    with tile.TileContext(nc) as tc, ExitStack() as ctx:
        const = ctx.enter_context(tc.tile_pool(name="const", bufs=1))
        qk_pool = ctx.enter_context(tc.tile_pool(name="qkp", bufs=2))
        v_pool = ctx.enter_context(tc.tile_pool(name="vp", bufs=2))
        pt_pool = ctx.enter_context(tc.tile_pool(name="ptp", bufs=3))
        nrm_pool = ctx.enter_context(tc.tile_pool(name="nrm", bufs=1))
        rb_pool = ctx.enter_context(tc.tile_pool(name="rbp", bufs=2))
        yu_pool = ctx.enter_context(tc.tile_pool(name="yup", bufs=2))
        st_pool = ctx.enter_context(tc.tile_pool(name="stp", bufs=2))
        # PSUM: S tiles (2 banks x 2 bufs) | Y accumulator (2 banks) | qkv/
        # vtr/proj rotating pool (2 banks)
        ps_s = ctx.enter_context(tc.tile_pool(name="pss", bufs=2, space="PSUM"))
        ps_y = ctx.enter_context(tc.tile_pool(name="psy", bufs=1, space="PSUM"))
        ps_q = ctx.enter_context(tc.tile_pool(name="psq", bufs=2, space="PSUM"))

        # ---- constants ----
        wqkv_sb = const.tile([128, CT, 3 * RPC], bf16, tag="wqkv")
        nc.sync.dma_start(out=wqkv_sb, in_=wqkv.rearrange("ct p r -> p ct r"))
        wproj_sb = const.tile([128, C], bf16, tag="wproj")
        nc.sync.dma_start(out=wproj_sb, in_=wproj)

        ident = const.tile([128, 128], bf16, tag="ident")
        from concourse.masks import make_identity
        make_identity(nc, ident)

        # stage all of x^T in SBUF once (128KB/partition) - each region is
        # written exactly once so no DMA ever carries a WAR/WAW wait.
        xt_sb = const.tile([128, B, CT, T], bf16, tag="xts")
        for b in range(B):
            for c in range(CT):
                nc.sync.dma_start(out=xt_sb[:, b, c, :], in_=xt[c, b])

        # tri[p, f] = 1.0 where p <= f else 0 (keep k <= q in transposed scores)
        tri = const.tile([128, 128], bf16, tag="tri")
        nc.gpsimd.memset(tri, 1.0)
        nc.gpsimd.affine_select(
            out=tri, in_=tri,
            compare_op=mybir.AluOpType.is_ge,
            fill=0.0, base=0,
            channel_multiplier=-1,       # expr = -p + f >= 0  -> keep
            pattern=[[1, 128]],
        )

        st = {}   # per-b tiles

        def _make_tiles(b):
            qt_b = qk_pool.tile([128, T], bf16, tag="qt", name=f"qt_{b}")
            kt_b = qk_pool.tile([128, T], bf16, tag="kt", name=f"kt_{b}")
            vt_b = qk_pool.tile([128, T], bf16, tag="vt", name=f"vt_{b}")
            yl_b = qk_pool.tile([128, T], bf16, tag="yl", name=f"yl_{b}")
            v_b = v_pool.tile([128, NTT, HPC, 128], bf16, tag="v",
                              name=f"v_{b}")
            # h0: hd at cols 0:64, ones at col 64, zeros above.
            # h1: ones at col 0 (partition-aligned l row), hd at 64:128.
            nc.vector.memset(v_b[:, :, 0, HD:], 0.0)
            nc.vector.memset(v_b[:, :, 0, HD:HD + 1], 1.0)
            nc.vector.memset(v_b[:, :, 1, :HD], 0.0)
            nc.vector.memset(v_b[:, :, 1, 0:1], 1.0)
            st[b] = (qt_b, kt_b, vt_b, yl_b, v_b)

        def emit_qkv_unit(b, q5):
            """QKV projection for one 512-col t-chunk + V transposes for it."""
            if b not in st:
                _make_tiles(b)
            qt_b, kt_b, vt_b, yl_b, v_b = st[b]
            for rg, dest in ((0, qt_b), (1, kt_b), (2, vt_b)):
                acc = ps_q.tile([128, 512], f32, tag="q",
                                name=f"qkv_{b}_{rg}_{q5}")
                for c in range(CT):
                    nc.tensor.matmul(
                        acc,
                        lhsT=wqkv_sb[:, c, rg * 128:(rg + 1) * 128],
                        rhs=xt_sb[:, b, c, q5 * 512:(q5 + 1) * 512],
                        start=(c == 0), stop=(c == CT - 1),
                    )
                nc.vector.tensor_copy(dest[:, q5 * 512:(q5 + 1) * 512], acc)
            for tt in range(4 * q5, 4 * q5 + 4):
                vtr = ps_q.tile([128, 128], bf16, tag="q", name=f"vtr_{b}_{tt}")
                nc.tensor.transpose(vtr, vt_b[:, tt * 128:(tt + 1) * 128],
                                    ident)
                nc.vector.tensor_copy(v_b[:, tt, 0, 0:HD], vtr[:, 0:HD])
                nc.vector.tensor_copy(v_b[:, tt, 1, HD:128], vtr[:, HD:128])

        def emit_attn_chunk(b, h, ch):
            """Causal attention for head h, q-chunk ch (1024 wide)."""
            qt_b, kt_b, vt_b, yl_b, v_b = st[b]
            hp = h * HD
            q0 = ch * QCH
            nkt = 8 * (ch + 1)
            alast = 3 + 8 * ch        # last j contributing to cols [0:512)
            yaug = ps_y.tile([128, QCH], f32, tag="y", name=f"yaug_{b}_{h}_{ch}")

            def consume(j, s, lo):
                p = pt_pool.tile([128, QCH], bf16, tag="p",
                                 name=f"p_{b}_{h}_{ch}_{j}")
                nc.scalar.activation(p[:, lo:QCH], s[:, lo:QCH], EXP,
                                     scale=1.0 / 8.0)
                dl = 128 * j - QCH * ch
                if dl >= 0:   # diagonal block [dl, dl+128) needs causal mask
                    nc.vector.tensor_mul(p[:, dl:dl + 128], p[:, dl:dl + 128],
                                         tri)
                if lo < 512:
                    nc.tensor.matmul(
                        yaug[:, lo:512],
                        lhsT=v_b[:, j, h, :],
                        rhs=p[:, lo:512],
                        start=(j == 0), stop=(j == alast),
                    )
                b0 = max(lo, 512)
                nc.tensor.matmul(
                    yaug[:, b0:QCH],
                    lhsT=v_b[:, j, h, :],
                    rhs=p[:, b0:QCH],
                    start=(j == 0), stop=(j == nkt - 1),
                )

            prev = None
            for j in range(nkt):
                lo = max(0, 128 * j - QCH * ch)
                s = ps_s.tile([128, QCH], f32, tag="s",
                              name=f"s_{b}_{h}_{ch}_{j}")
                if lo < 512:
                    nc.tensor.matmul(
                        s[:, lo:512],
                        lhsT=kt_b[hp:hp + HD, j * 128:(j + 1) * 128],
                        rhs=qt_b[hp:hp + HD, q0 + lo:q0 + 512],
                        start=True, stop=True,
                    )
                b0 = max(lo, 512)
                nc.tensor.matmul(
                    s[:, b0:QCH],
                    lhsT=kt_b[hp:hp + HD, j * 128:(j + 1) * 128],
                    rhs=qt_b[hp:hp + HD, q0 + b0:q0 + QCH],
                    start=True, stop=True,
                )
                if prev is not None:
                    consume(*prev)
                prev = (j, s, lo)
            consume(*prev)

            # Evacuate yaug early (one full-height CAST frees both PSUM banks
            # in ~1.2us), then normalize entirely off the PE/ACT engines:
            # broadcast raw l with GPSIMD, 1/l via int bit-trick + one
            # Newton step on DVE, multi-lane.
            yu = yu_pool.tile([128, QCH], bf16, tag="yu",
                              name=f"yu_{b}_{h}_{ch}")
            nc.vector.tensor_copy(yu, yaug)
            rb_sb = rb_pool.tile([128, QCH], bf16, tag="rb",
                                 name=f"rb_{b}_{h}_{ch}")
            if h == 0:
                # l sits at partition 64; partition_broadcast needs src at
                # partition 0 -> tiny SBUF->SBUF DMA row shift first.
                nc.sync.dma_start(out=rb_sb[0:1, :], in_=yu[HD:HD + 1, :])
                nc.gpsimd.partition_broadcast(rb_sb[0:HD, :], rb_sb[0:1, :])
                rows = slice(0, HD)
            else:
                nc.gpsimd.partition_broadcast(rb_sb, yu[0:1, :])
                rows = slice(HD, 128)
            i16 = mybir.dt.int16
            y0 = nrm_pool.tile([128, QCH], bf16, tag="y0",
                               name=f"y0_{b}_{h}_{ch}")
            nc.vector.tensor_scalar(
                y0[rows, :].bitcast(i16), rb_sb[rows, :].bitcast(i16),
                -1.0, float(0x7EF7), mybir.AluOpType.mult, mybir.AluOpType.add)
            tn = nrm_pool.tile([128, QCH], f16, tag="tn",
                               name=f"tn_{b}_{h}_{ch}")
            nc.vector.tensor_mul(tn[rows, :], y0[rows, :], rb_sb[rows, :])
            nc.vector.tensor_scalar(
                tn[rows, :], tn[rows, :], -1.0, 2.0,
                mybir.AluOpType.mult, mybir.AluOpType.add)
            nc.vector.tensor_mul(rb_sb[rows, :], y0[rows, :], tn[rows, :])
            nc.vector.tensor_mul(yl_b[hp:hp + HD, q0:q0 + QCH],
                                 yu[rows, :], rb_sb[rows, :])

        def emit_proj(b, ch):
            """Output projection for the 8 token-tiles of q-chunk ch."""
            yl_b = st[b][3]
            for tt in range(8 * ch, 8 * ch + 8):
                for n5 in range(C // 512):
                    op = ps_q.tile([128, 512], f32, tag="q",
                                   name=f"op_{b}_{tt}_{n5}")
                    nc.tensor.matmul(
                        op,
                        lhsT=yl_b[:, tt * 128:(tt + 1) * 128],
                        rhs=wproj_sb[:, n5 * 512:(n5 + 1) * 512],
                        start=True, stop=True,
                    )
                    o_sb = st_pool.tile([128, 512], bf16, tag="o",
                                        name=f"o_{b}_{tt}_{n5}")
                    if tt % 4 == 3:
                        nc.scalar.copy(o_sb, op)
                    else:
                        nc.vector.tensor_copy(o_sb, op)
                    nc.sync.dma_start(
                        out=outp[b * T + tt * 128: b * T + (tt + 1) * 128,
                                 n5 * 512:(n5 + 1) * 512],
                        in_=o_sb)

        # software pipeline: QKV of b+1 and proj of b interleave with attn of b
        for q5 in range(4):
            emit_qkv_unit(0, q5)
        for b in range(B):
            fill = iter(range(4))
            for h in range(HPC):
                for ch in range(NCH):
                    emit_attn_chunk(b, h, ch)
                    if b + 1 < B:
                        emit_qkv_unit(b + 1, next(fill))
                    if h == 1:
                        emit_proj(b, ch)
            del st[b]

    nc.compile()
    return nc


def _prep_inputs(x, w_attn, w_proj):
    """Host-side sharding: build per-core input maps."""
    bf16 = ml_dtypes.bfloat16
    x = np.asarray(x, dtype=np.float32)
    w_attn = np.asarray(w_attn, dtype=np.float32)
    w_proj = np.asarray(w_proj, dtype=np.float32)

    # x^T tiles: [CT, B, 128, T]
    xt = np.ascontiguousarray(
        x.reshape(BT, C).T.reshape(CT, 128, B, T).transpose(0, 2, 1, 3)
    ).astype(bf16)

    in_maps = []
    for g in range(NCORES):
        r0 = g * RPC
        w_local = np.concatenate([
            w_attn[r0:r0 + RPC],              # q rows of heads 2g, 2g+1
            w_attn[C + r0:C + r0 + RPC],      # k rows
            w_attn[2 * C + r0:2 * C + r0 + RPC],  # v rows
        ], axis=0)                            # [384, C]
        wqkv = np.ascontiguousarray(
            w_local.T.reshape(CT, 128, 3 * RPC)).astype(bf16)
        wprojT = np.ascontiguousarray(w_proj[:, r0:r0 + RPC].T).astype(bf16)
        in_maps.append({"xt": xt, "wqkv": wqkv, "wproj": wprojT})
    return in_maps


def kernel(x, w_attn, w_proj):
    from concourse import bass_utils

    if "nc" not in _prog_cache:
        _prog_cache["nc"] = build_program()
    nc = _prog_cache["nc"]

    in_maps = _prep_inputs(x, w_attn, w_proj)
    res = bass_utils.run_bass_kernel_spmd(
        nc, in_maps, core_ids=list(range(NCORES)))

    acc = np.zeros((BT, C), dtype=np.float32)
    for g in range(NCORES):
        part = np.asarray(res.results[g]["outp"])
        if part.dtype != np.float32:
            # bf16 -> f32 exact upcast via bit manipulation (fast on host)
            part = (part.view(np.uint16).astype(np.uint32) << 16).view(np.float32)
        acc += part
    return acc.reshape(B, T, C)
